# revision 3
# baseline (speedup 1.0000x reference)
"""MaxUnpooling2D scatter-add kernel for Trainium2 (8 NeuronCores).

Reference semantics (per batch b):
    y = mask // (OW*C); x = (mask // C) % OW; f = channel index c
    out[b, y, x, c] += updates[b, h, w, c]      (duplicates sum)

Strategy (pure data-parallel over batch; 2 batches per core):
  - Layout SBUF tiles [128 partitions, 4096] where partition p holds hw rows
    [32p, 32p+32) and free column j = q*128 + c  (q in [0,32), c = channel).
  - For each (plane c, chunk q): the 128 elements (one per partition) are
    scatter-routed with a dense one-hot matmul:
        A[i, y]  = (iota == Y[i])             (stationary operand)
        Bv[i, x] = (iota == X[i]) * V[i]      (moving operand)
        psum_c[y, x] += A.T @ Bv              (PE contraction over i)
    PSUM (f32) accumulates the 32 chunks of a plane; duplicates sum exactly.
  - Evacuate psum_c[y, x] into PL[y, x, c]; one contiguous 8MB DMA per batch.

V2 engine split:
  - A (stationary) one-hot: plane-batched DVE tensor_tensor is_equal in the
    [p, y, q] layout with materialized iotaT (2x packed mode). LDWEIGHTS
    reads the strided [p, :, q] slice (~147ns, overlapped).
  - Bv (moving) one-hot: ONE chunk-granular DVE tensor_scalar per (c, q):
    out[i, x] = (iota_w[i, x] == X[i]) * V[i] with per-partition scalar
    operands X=xtr[:,c,q], V=vtr[:,c,q]. Scalar operands are exempt from
    the packed-mode stride checks, so this runs in the DVE 4x mode
    (~94ns/chunk) AND produces a fully contiguous [128,128] fp16 moving
    tile, fixing the strided-moving matmul penalty (was 410ns/matmul).
  - PSUM evac on ACT. fp16 one-hots/values (lane ids exact; values rounded
    to 11 bits -> ~2e-4 rel err); PSUM accumulates in f32.
"""

import sys

sys.path.insert(0, "/opt/trn_rl_repo")

import numpy as np

import concourse.bacc as bacc
import concourse.bass as bass
import concourse.tile as tile
from concourse import mybir
from concourse.bass_utils import run_bass_kernel_spmd

# Problem shape (hardcoded per contract)
B, H, W, C = 16, 64, 64, 128
OH, OW = 2 * H, 2 * W
N_CORES = 8
B_PER_CORE = B // N_CORES  # 2
HWF = H * W  # 4096
P = 128
Q = HWF // P  # 32 hw rows per partition
NCOL = Q * C  # 4096

F32 = mybir.dt.float32
BF16 = mybir.dt.bfloat16
FP16 = mybir.dt.float16
I32 = mybir.dt.int32

def build_nc(n_planes=C, dt=FP16):
    nc = bacc.Bacc("TRN2", target_bir_lowering=False, debug=False)

    upd = nc.declare_dram_parameter("updates", [B_PER_CORE, HWF, C], F32, isOutput=False)
    msk = nc.declare_dram_parameter("mask", [B_PER_CORE, HWF, C], I32, isOutput=False)
    iota_in = nc.declare_dram_parameter("iota", [P, P], F32, isOutput=False)
    out = nc.declare_dram_parameter("out", [B_PER_CORE, OH, OW, C], F32, isOutput=True)

    with tile.TileContext(nc) as tc:
        with (
            tc.tile_pool(name="const", bufs=1) as const_pool,
            tc.tile_pool(name="inp", bufs=1) as inp_pool,
            tc.tile_pool(name="pl", bufs=1) as pl_pool,
            tc.tile_pool(name="apool", bufs=3) as a_pool,
            tc.tile_pool(name="bpool", bufs=8) as b_pool,
            tc.tile_pool(name="psum", bufs=8, space="PSUM") as psum_pool,
        ):
            iota_f = const_pool.tile([P, P], F32)
            nc.sync.dma_start(iota_f[:], iota_in[:])
            # materialized iotaT[p, y, q] = y  (fp16, innermost step 1) so the
            # plane-batched A build qualifies for the DVE 2x packed mode
            iotaT = const_pool.tile([P, P, Q], dt)
            nc.vector.tensor_copy(
                iotaT[:],
                iota_f[:]
                .rearrange("p (y o) -> p y o", o=1)
                .broadcast_to([P, P, Q]),
            )
            # iota along the free dim [p, w] = w, fp16 contiguous — in0 of the
            # chunk-granular Bv tensor_scalar builds
            iota_w = const_pool.tile([P, P], dt)
            nc.vector.tensor_copy(iota_w[:], iota_f[:])

            for b in range(B_PER_CORE):
                # ---- load batch b ----
                u_f = inp_pool.tile([P, NCOL], F32, tag="uf")
                nc.sync.dma_start(u_f[:], upd[b].rearrange("(p q) c -> p (q c)", p=P))
                m = inp_pool.tile([P, NCOL], I32, tag="m")
                nc.sync.dma_start(m[:], msk[b].rearrange("(p q) c -> p (q c)", p=P))

                # ---- decode mask -> channel-major fp16 Y/X/V tiles [p, c, q] ----
                yi = inp_pool.tile([P, NCOL], I32, tag="yi")
                nc.vector.tensor_scalar(
                    yi[:], m[:], 14, None, mybir.AluOpType.logical_shift_right
                )
                ytr = inp_pool.tile([P, C, Q], dt, tag="ytr")
                nc.vector.tensor_copy(ytr[:], yi[:].rearrange("p (q c) -> p c q", c=C))

                xi = inp_pool.tile([P, NCOL], I32, tag="yi")
                nc.vector.tensor_scalar(
                    xi[:],
                    m[:],
                    7,
                    127,
                    mybir.AluOpType.logical_shift_right,
                    mybir.AluOpType.bitwise_and,
                )
                # X/V as f32: tensor_scalar's scalar operands must be f32 for
                # is_equal; scalar operands are exempt from packed-mode checks
                xtr = inp_pool.tile([P, C, Q], F32, tag="xtr")
                nc.vector.tensor_copy(xtr[:], xi[:].rearrange("p (q c) -> p c q", c=C))

                vtr = inp_pool.tile([P, C, Q], F32, tag="vtr")
                nc.vector.tensor_copy(vtr[:], u_f[:].rearrange("p (q c) -> p c q", c=C))

                pl = pl_pool.tile([P, P, C], F32)  # [y, x, c]
                if n_planes < C:
                    nc.gpsimd.memset(pl[:], 0.0)

                for c in range(n_planes):
                    # plane-batched A build (DVE 2x): a[p, y, q] = (iotaT == Y)
                    a_pl = a_pool.tile([P, P, Q], dt, tag="a")
                    y_bc = (
                        ytr[:, c, :]
                        .rearrange("p (o q) -> p o q", o=1)
                        .broadcast_to([P, P, Q])
                    )
                    nc.vector.tensor_tensor(
                        a_pl[:], iotaT[:], y_bc, mybir.AluOpType.is_equal
                    )

                    acc = psum_pool.tile([P, P], F32)  # [y, x]
                    for q in range(Q):
                        # chunk Bv build (DVE 4x, contiguous): one fused
                        # tensor_scalar: (iota_w == X[i]) * V[i]
                        b_ch = b_pool.tile([P, P], dt, tag="b")
                        nc.vector.tensor_scalar(
                            b_ch[:],
                            iota_w[:],
                            xtr[:, c, q : q + 1],
                            vtr[:, c, q : q + 1],
                            mybir.AluOpType.is_equal,
                            mybir.AluOpType.mult,
                        )
                        # psum[y, x] += sum_i a[i, y] * b[i, x]
                        nc.tensor.matmul(
                            acc[:],
                            a_pl[:, :, q],
                            b_ch[:],
                            start=(q == 0),
                            stop=(q == Q - 1),
                        )
                    # evacuate plane: pl[:, :, c] = acc
                    nc.scalar.copy(pl[:, :, c], acc[:])

                nc.sync.dma_start(out[b].rearrange("y x c -> y (x c)"), pl[:])

    nc.compile()
    return nc


_CACHED = {}


def _get_nc(n_planes=C):
    key = n_planes
    if key not in _CACHED:
        _CACHED[key] = build_nc(n_planes)
    return _CACHED[key]


def kernel(updates: np.ndarray, mask: np.ndarray) -> np.ndarray:
    nc = _get_nc()
    iota = np.broadcast_to(np.arange(P, dtype=np.float32), (P, P)).copy()
    in_maps = []
    for i in range(N_CORES):
        sl = slice(i * B_PER_CORE, (i + 1) * B_PER_CORE)
        in_maps.append(
            {
                "updates": np.ascontiguousarray(
                    updates[sl].reshape(B_PER_CORE, HWF, C), dtype=np.float32
                ),
                "mask": np.ascontiguousarray(
                    mask[sl].reshape(B_PER_CORE, HWF, C), dtype=np.int32
                ),
                "iota": iota,
            }
        )
    res = run_bass_kernel_spmd(nc, in_maps, list(range(N_CORES)))
    return np.concatenate([res.results[i]["out"] for i in range(N_CORES)], axis=0)


# revision 11
# speedup vs baseline: 1.0466x; 1.0466x over previous
"""MaxUnpooling2D scatter-add kernel for Trainium2 (8 NeuronCores).

Reference semantics (per batch b):
    y = mask // (OW*C); x = (mask // C) % OW; f = channel index c
    out[b, y, x, c] += updates[b, h, w, c]      (duplicates sum)

Strategy (pure data-parallel over batch; 2 batches per core):
  - SBUF input layout [128 partitions, 4096]: partition p holds hw rows
    [32p, 32p+32), free column j = q*128 + c  (q in [0,32), c = channel).
  - Per (plane c, chunk q): 128 elements (one per partition) scatter-routed
    with a dense one-hot matmul:
        A[i, y]  = (iota == Y[i])             (stationary)
        Bv[i, x] = (iota == X[i]) * V[i]      (moving)
        psum_c[y, x] += A.T @ Bv              (PE contraction over i)
    PSUM f32 accumulates the 32 chunks of a plane; duplicates sum exactly.

V4 engine split (DVE was the 89-99% bottleneck; spread the 3 build ops):
  - A-eq and X-eq: plane-batched DVE tensor_tensor is_equal (2x packed) in
    the q-interleaved layout [p, qh=16, w=128, ql=2] (chunk q = 2qh+ql).
    Innermost dim is the ql pair -> 2x stays; matmul operand slices
    [p, qh, :, ql] have 4-byte w-stride (vs 64B in the plain layout).
  - V-mult split per c%16: 1 plane DVE (batched fp16 mult), 6 planes ACT
    (32 chunk ops: activation Copy with per-partition scale = V column
    sliced straight from the f32 input tile; writes CONTIGUOUS [128,128]
    moving tiles), 9 planes Pool (batched gpsimd mult, ~2x DVE cycle cost
    but otherwise-idle engine).
  - Matmul chains for a PAIR of planes interleave chunk-by-chunk so
    consecutive PE matmuls alternate PSUM banks (hides the same-bank
    accumulation turnaround); PSUM evac on ACT, deferred one pair so ACT
    pre-builds the next pair's chunk-mults first.
"""

import sys

sys.path.insert(0, "/opt/trn_rl_repo")

import numpy as np

import concourse.bacc as bacc
import concourse.bass as bass
import concourse.tile as tile
from concourse import mybir
from concourse.bass_utils import run_bass_kernel_spmd

# Problem shape (hardcoded per contract)
B, H, W, C = 16, 64, 64, 128
OH, OW = 2 * H, 2 * W
N_CORES = 8
B_PER_CORE = B // N_CORES  # 2
HWF = H * W  # 4096
P = 128
Q = HWF // P  # 32 hw rows per partition
Q2 = Q // 2  # 16 interleaved chunk pairs
NCOL = Q * C  # 4096

F32 = mybir.dt.float32
BF16 = mybir.dt.bfloat16
FP16 = mybir.dt.float16
I32 = mybir.dt.int32

# V-mult engine assignment per plane index mod 16
_ACT_SET = frozenset((1, 3, 5, 7, 9, 11))
_DVE_SET = frozenset((0,))


def _mult_engine(c):
    r = c % 16
    if r in _DVE_SET:
        return "dve"
    if r in _ACT_SET:
        return "act"
    return "pool"


def build_nc(n_planes=C, dt=FP16):
    nc = bacc.Bacc("TRN2", target_bir_lowering=False, debug=False)

    upd = nc.declare_dram_parameter("updates", [B_PER_CORE, HWF, C], F32, isOutput=False)
    msk = nc.declare_dram_parameter("mask", [B_PER_CORE, HWF, C], I32, isOutput=False)
    iota_in = nc.declare_dram_parameter("iota", [P, P], F32, isOutput=False)
    out = nc.declare_dram_parameter("out", [B_PER_CORE, OH, OW, C], F32, isOutput=True)

    with tile.TileContext(nc) as tc:
        with (
            tc.tile_pool(name="const", bufs=1) as const_pool,
            tc.tile_pool(name="inp", bufs=1) as inp_pool,
            tc.tile_pool(name="pl", bufs=1) as pl_pool,
            tc.tile_pool(name="apool", bufs=3) as a_pool,
            tc.tile_pool(name="xpool", bufs=2) as x_pool,
            tc.tile_pool(name="bpool", bufs=3) as b_pool,
            tc.tile_pool(name="bch", bufs=8) as bch_pool,
            tc.tile_pool(name="psum", bufs=8, space="PSUM") as psum_pool,
        ):
            iota_f = const_pool.tile([P, P], F32)
            nc.sync.dma_start(iota_f[:], iota_in[:])
            # materialized iota2[p, w, ql] = w (fp16, innermost ql step 1)
            iota2 = const_pool.tile([P, P, 2], dt)
            nc.vector.tensor_copy(
                iota2[:],
                iota_f[:]
                .rearrange("p (w o) -> p w o", o=1)
                .broadcast_to([P, P, 2]),
            )

            def bc_in0():
                return (
                    iota2[:]
                    .rearrange("p (o w) ql -> p o w ql", o=1)
                    .broadcast_to([P, Q2, P, 2])
                )

            def bc_col(tile_, c):
                return (
                    tile_[:, c, :]
                    .rearrange("p (qh o ql) -> p qh o ql", o=1, ql=2)
                    .broadcast_to([P, Q2, P, 2])
                )

            for b in range(B_PER_CORE):
                # ---- load batch b ----
                u_f = inp_pool.tile([P, NCOL], F32, tag="uf")
                nc.sync.dma_start(u_f[:], upd[b].rearrange("(p q) c -> p (q c)", p=P))
                m = inp_pool.tile([P, NCOL], I32, tag="m")
                nc.sync.dma_start(m[:], msk[b].rearrange("(p q) c -> p (q c)", p=P))

                # ---- decode mask -> channel-major fp16 Y/X/V tiles [p, c, q] ----
                yi = inp_pool.tile([P, NCOL], I32, tag="yi")
                nc.vector.tensor_scalar(
                    yi[:], m[:], 14, None, mybir.AluOpType.logical_shift_right
                )
                ytr = inp_pool.tile([P, C, Q], dt, tag="ytr")
                nc.vector.tensor_copy(ytr[:], yi[:].rearrange("p (q c) -> p c q", c=C))

                xi = inp_pool.tile([P, NCOL], I32, tag="yi")
                nc.vector.tensor_scalar(
                    xi[:],
                    m[:],
                    7,
                    127,
                    mybir.AluOpType.logical_shift_right,
                    mybir.AluOpType.bitwise_and,
                )
                xtr = inp_pool.tile([P, C, Q], dt, tag="xtr")
                nc.vector.tensor_copy(xtr[:], xi[:].rearrange("p (q c) -> p c q", c=C))

                vtr = inp_pool.tile([P, C, Q], dt, tag="vtr")
                nc.vector.tensor_copy(vtr[:], u_f[:].rearrange("p (q c) -> p c q", c=C))

                pl = pl_pool.tile([P, P, C], F32)  # [y, x, c]
                if n_planes < C:
                    nc.gpsimd.memset(pl[:], 0.0)

                pending_evac = None
                for cp in range(0, n_planes, 2):
                    pair = [cp, cp + 1] if cp + 1 < n_planes else [cp]
                    movings, accs = [], []
                    for c in pair:
                        # A build [p, qh, w, ql] = (iota == Y)  (DVE 2x)
                        a2 = a_pool.tile([P, Q2, P, 2], dt, tag="a")
                        nc.vector.tensor_tensor(
                            a2[:], bc_in0(), bc_col(ytr, c), mybir.AluOpType.is_equal
                        )
                        # X-eq (DVE 2x)
                        xeq = x_pool.tile([P, Q2, P, 2], dt, tag="xeq")
                        nc.vector.tensor_tensor(
                            xeq[:], bc_in0(), bc_col(xtr, c), mybir.AluOpType.is_equal
                        )

                        eng = _mult_engine(c)
                        if eng == "act":
                            # 32 chunk mults on ACT -> contiguous moving tiles
                            chunks = []
                            for q in range(Q):
                                qh, ql = q // 2, q % 2
                                b_ch = bch_pool.tile([P, P], dt, tag="bch")
                                nc.scalar.activation(
                                    b_ch[:],
                                    xeq[:, qh, :, ql],
                                    mybir.ActivationFunctionType.Copy,
                                    bias=0.0,
                                    scale=u_f[:, q * C + c : q * C + c + 1],
                                )
                                chunks.append(b_ch)
                            movings.append(
                                lambda qh, ql, ch=chunks: ch[2 * qh + ql][:]
                            )
                        else:
                            b2 = b_pool.tile([P, Q2, P, 2], dt, tag="b")
                            e = nc.vector if eng == "dve" else nc.gpsimd
                            e.tensor_tensor(
                                b2[:], xeq[:], bc_col(vtr, c), mybir.AluOpType.mult
                            )
                            movings.append(
                                lambda qh, ql, t=b2: t[:, qh, :, ql]
                            )
                        acc = psum_pool.tile([P, P], F32, tag="acc")  # [y, x]
                        accs.append((acc, a2))

                    # interleaved accumulation chains across the pair's banks
                    for qh in range(Q2):
                        for ql in range(2):
                            for k in range(len(pair)):
                                acc, a2 = accs[k]
                                nc.tensor.matmul(
                                    acc[:],
                                    a2[:, qh, :, ql],
                                    movings[k](qh, ql),
                                    start=(qh == 0 and ql == 0),
                                    stop=(qh == Q2 - 1 and ql == 1),
                                )

                    # deferred evac: ACT drains the PREVIOUS pair after it has
                    # issued this pair's chunk mults (keeps ACT pipelined)
                    if pending_evac is not None:
                        for acc_, c_ in pending_evac:
                            nc.scalar.copy(pl[:, :, c_], acc_[:])
                    pending_evac = [(accs[k][0], pair[k]) for k in range(len(pair))]

                if pending_evac is not None:
                    for acc_, c_ in pending_evac:
                        nc.scalar.copy(pl[:, :, c_], acc_[:])

                nc.sync.dma_start(out[b].rearrange("y x c -> y (x c)"), pl[:])

    nc.compile()
    return nc


_CACHED = {}


def _get_nc(n_planes=C):
    key = n_planes
    if key not in _CACHED:
        _CACHED[key] = build_nc(n_planes)
    return _CACHED[key]


def kernel(updates: np.ndarray, mask: np.ndarray) -> np.ndarray:
    nc = _get_nc()
    iota = np.broadcast_to(np.arange(P, dtype=np.float32), (P, P)).copy()
    in_maps = []
    for i in range(N_CORES):
        sl = slice(i * B_PER_CORE, (i + 1) * B_PER_CORE)
        in_maps.append(
            {
                "updates": np.ascontiguousarray(
                    updates[sl].reshape(B_PER_CORE, HWF, C), dtype=np.float32
                ),
                "mask": np.ascontiguousarray(
                    mask[sl].reshape(B_PER_CORE, HWF, C), dtype=np.int32
                ),
                "iota": iota,
            }
        )
    res = run_bass_kernel_spmd(nc, in_maps, list(range(N_CORES)))
    return np.concatenate([res.results[i]["out"] for i in range(N_CORES)], axis=0)


# revision 13
# speedup vs baseline: 1.1878x; 1.1349x over previous
"""MaxUnpooling2D scatter-add kernel for Trainium2 (8 NeuronCores).

Reference semantics (per batch b):
    y = mask // (OW*C); x = (mask // C) % OW; f = channel index c
    out[b, y, x, c] += updates[b, h, w, c]      (duplicates sum)

Strategy (pure data-parallel over batch; 2 batches per core):
  - SBUF input layout [128 partitions, 4096]: partition p holds hw rows
    [32p, 32p+32), free column j = q*128 + c  (q in [0,32), c = channel).
  - Per (plane c, chunk q): 128 elements (one per partition) scatter-routed
    with a dense one-hot matmul:
        A[i, y]  = (iota == Y[i])             (stationary)
        Bv[i, x] = (iota == X[i]) * V[i]      (moving)
        psum_c[y, x] += A.T @ Bv              (PE contraction over i)
    PSUM f32 accumulates the 32 chunks of a plane; duplicates sum exactly.

V5 specifics (V4 postmortem: 3-way mult split caused dependency stalls —
no engine above 65%; keep the PE wins, simplify the offload):
  - One-hot tiles q-interleaved [p, qh=16, w=128, ql=2] (chunk q = 2qh+ql):
    builds stay DVE-2x (innermost ql pair stride 1), matmul operand slices
    [p, qh, :, ql] get a 4-byte w-stride (mm ~308ns vs 410ns at 64B).
  - Matmul chains for a PAIR of planes interleave chunk-by-chunk so
    consecutive PE matmuls alternate PSUM banks (hides same-bank
    accumulation turnaround; mm p10 219ns).
  - A-eq + X-eq + most V-mults on DVE (batched 2x). Planes with c%8 in
    {2,3} (one homogeneous pair per 8) send the V-mult to the otherwise
    idle Pool engine (batched gpsimd mult ~8.9us), V read straight from
    the f32 input tile (no fp16 vtr copy).
  - PSUM evac on ACT, deferred one pair for pipelining.
"""

import sys

sys.path.insert(0, "/opt/trn_rl_repo")

import numpy as np

import concourse.bacc as bacc
import concourse.bass as bass
import concourse.tile as tile
from concourse import mybir
from concourse.bass_utils import run_bass_kernel_spmd

# Problem shape (hardcoded per contract)
B, H, W, C = 16, 64, 64, 128
OH, OW = 2 * H, 2 * W
N_CORES = 8
B_PER_CORE = B // N_CORES  # 2
HWF = H * W  # 4096
P = 128
Q = HWF // P  # 32 hw rows per partition
Q2 = Q // 2  # 16 interleaved chunk pairs
NCOL = Q * C  # 4096

F32 = mybir.dt.float32
BF16 = mybir.dt.bfloat16
FP16 = mybir.dt.float16
I32 = mybir.dt.int32

# planes whose V-mult runs on Pool (one homogeneous PAIR per 8 planes)
_POOL_SET = frozenset((2, 3))


def build_nc(n_planes=C, dt=FP16):
    nc = bacc.Bacc("TRN2", target_bir_lowering=False, debug=False)

    upd = nc.declare_dram_parameter("updates", [B_PER_CORE, HWF, C], F32, isOutput=False)
    msk = nc.declare_dram_parameter("mask", [B_PER_CORE, HWF, C], I32, isOutput=False)
    iota_in = nc.declare_dram_parameter("iota", [P, P], F32, isOutput=False)
    out = nc.declare_dram_parameter("out", [B_PER_CORE, OH, OW, C], F32, isOutput=True)

    with tile.TileContext(nc) as tc:
        with (
            tc.tile_pool(name="const", bufs=1) as const_pool,
            tc.tile_pool(name="inp", bufs=1) as inp_pool,
            tc.tile_pool(name="pl", bufs=1) as pl_pool,
            tc.tile_pool(name="apool", bufs=3) as a_pool,
            tc.tile_pool(name="xpool", bufs=2) as x_pool,
            tc.tile_pool(name="bpool", bufs=3) as b_pool,
            tc.tile_pool(name="psum", bufs=8, space="PSUM") as psum_pool,
        ):
            iota_f = const_pool.tile([P, P], F32)
            nc.sync.dma_start(iota_f[:], iota_in[:])
            # materialized iota2[p, w, ql] = w (fp16, innermost ql step 1)
            iota2 = const_pool.tile([P, P, 2], dt)
            nc.vector.tensor_copy(
                iota2[:],
                iota_f[:]
                .rearrange("p (w o) -> p w o", o=1)
                .broadcast_to([P, P, 2]),
            )

            def bc_in0():
                return (
                    iota2[:]
                    .rearrange("p (o w) ql -> p o w ql", o=1)
                    .broadcast_to([P, Q2, P, 2])
                )

            def bc_col(tile_, c):
                return (
                    tile_[:, c, :]
                    .rearrange("p (qh o ql) -> p qh o ql", o=1, ql=2)
                    .broadcast_to([P, Q2, P, 2])
                )

            for b in range(B_PER_CORE):
                # ---- load batch b ----
                u_f = inp_pool.tile([P, NCOL], F32, tag="uf")
                nc.sync.dma_start(u_f[:], upd[b].rearrange("(p q) c -> p (q c)", p=P))
                m = inp_pool.tile([P, NCOL], I32, tag="m")
                nc.sync.dma_start(m[:], msk[b].rearrange("(p q) c -> p (q c)", p=P))

                # ---- decode mask -> channel-major fp16 Y/X/V tiles [p, c, q] ----
                yi = inp_pool.tile([P, NCOL], I32, tag="yi")
                nc.vector.tensor_scalar(
                    yi[:], m[:], 14, None, mybir.AluOpType.logical_shift_right
                )
                ytr = inp_pool.tile([P, C, Q], dt, tag="ytr")
                nc.vector.tensor_copy(ytr[:], yi[:].rearrange("p (q c) -> p c q", c=C))

                xi = inp_pool.tile([P, NCOL], I32, tag="yi")
                nc.vector.tensor_scalar(
                    xi[:],
                    m[:],
                    7,
                    127,
                    mybir.AluOpType.logical_shift_right,
                    mybir.AluOpType.bitwise_and,
                )
                xtr = inp_pool.tile([P, C, Q], dt, tag="xtr")
                nc.vector.tensor_copy(xtr[:], xi[:].rearrange("p (q c) -> p c q", c=C))

                vtr = inp_pool.tile([P, C, Q], dt, tag="vtr")
                nc.vector.tensor_copy(vtr[:], u_f[:].rearrange("p (q c) -> p c q", c=C))

                pl = pl_pool.tile([P, P, C], F32)  # [y, x, c]
                if n_planes < C:
                    nc.gpsimd.memset(pl[:], 0.0)

                pending_evac = None
                for cp in range(0, n_planes, 2):
                    pair = [cp, cp + 1] if cp + 1 < n_planes else [cp]
                    b2s, accs = [], []
                    for c in pair:
                        # A build [p, qh, w, ql] = (iota == Y)  (DVE 2x)
                        a2 = a_pool.tile([P, Q2, P, 2], dt, tag="a")
                        nc.vector.tensor_tensor(
                            a2[:], bc_in0(), bc_col(ytr, c), mybir.AluOpType.is_equal
                        )
                        # X-eq (DVE 2x)
                        xeq = x_pool.tile([P, Q2, P, 2], dt, tag="xeq")
                        nc.vector.tensor_tensor(
                            xeq[:], bc_in0(), bc_col(xtr, c), mybir.AluOpType.is_equal
                        )
                        # V-mult: Pool for c%8 in {2,3}, else DVE (batched 2x)
                        b2 = b_pool.tile([P, Q2, P, 2], dt, tag="b")
                        eng = nc.gpsimd if (c % 8) in _POOL_SET else nc.vector
                        eng.tensor_tensor(
                            b2[:], xeq[:], bc_col(vtr, c), mybir.AluOpType.mult
                        )
                        b2s.append(b2)
                        acc = psum_pool.tile([P, P], F32, tag="acc")  # [y, x]
                        accs.append((acc, a2))

                    # interleaved accumulation chains across the pair's banks
                    for qh in range(Q2):
                        for ql in range(2):
                            for k in range(len(pair)):
                                acc, a2 = accs[k]
                                nc.tensor.matmul(
                                    acc[:],
                                    a2[:, qh, :, ql],
                                    b2s[k][:, qh, :, ql],
                                    start=(qh == 0 and ql == 0),
                                    stop=(qh == Q2 - 1 and ql == 1),
                                )

                    # deferred evac keeps ACT a pair behind (pipelining)
                    if pending_evac is not None:
                        for acc_, c_ in pending_evac:
                            nc.scalar.copy(pl[:, :, c_], acc_[:])
                    pending_evac = [(accs[k][0], pair[k]) for k in range(len(pair))]

                if pending_evac is not None:
                    for acc_, c_ in pending_evac:
                        nc.scalar.copy(pl[:, :, c_], acc_[:])

                nc.sync.dma_start(out[b].rearrange("y x c -> y (x c)"), pl[:])

    nc.compile()
    return nc


_CACHED = {}


def _get_nc(n_planes=C):
    key = n_planes
    if key not in _CACHED:
        _CACHED[key] = build_nc(n_planes)
    return _CACHED[key]


def kernel(updates: np.ndarray, mask: np.ndarray) -> np.ndarray:
    nc = _get_nc()
    iota = np.broadcast_to(np.arange(P, dtype=np.float32), (P, P)).copy()
    in_maps = []
    for i in range(N_CORES):
        sl = slice(i * B_PER_CORE, (i + 1) * B_PER_CORE)
        in_maps.append(
            {
                "updates": np.ascontiguousarray(
                    updates[sl].reshape(B_PER_CORE, HWF, C), dtype=np.float32
                ),
                "mask": np.ascontiguousarray(
                    mask[sl].reshape(B_PER_CORE, HWF, C), dtype=np.int32
                ),
                "iota": iota,
            }
        )
    res = run_bass_kernel_spmd(nc, in_maps, list(range(N_CORES)))
    return np.concatenate([res.results[i]["out"] for i in range(N_CORES)], axis=0)
